# revision 56
# baseline (speedup 1.0000x reference)
"""GQA attention block (B=2, L=2048, D=4096, H=32, HKV=8, RoPE, causal) on 8
Trainium2 NeuronCores.

Sharding: core c -> batch b=c//4, head-group g=c%4 (8 Q heads + 2 KV heads per
core).  Each core computes x[b] @ wq_g/wk_g/wv_g projections, RoPE, causal
attention for its heads, and a partial output projection against its slice of
wo (row-sharded contraction).  The host sums the 4 partials per batch element.

All matmul operands are bf16 (fp32 PSUM accumulation): full PE rate with
fast-weight-load; the fp32r path runs in FP32_HIGH mode at ~60% throughput
with FWL disabled.  End-to-end relative error ~4.6e-3 (gate 2e-2).

Per 512-column l-chunk (4 chunks, in order, K/V tiles accumulate on chip):
  1. projections: 12 chains of 32 back-to-back matmuls accumulating the full
     D=4096 contraction in one PSUM bank.  V/K chains go first; their PSUM ->
     SBUF traffic runs on the scalar engine so the vector engine only does
     RoPE and the PE's V-transposes never wait behind queued RoPE ops.
     wk/wv stay resident in SBUF; wq streams per-chunk (2 tiles in flight); x
     streams per-chunk in four quarter-D tiles so the first chain starts
     after 1MB of DMA, not 4MB.
  2. causal attention, scores transposed S^T = K^T.T @ Q^T so probs feed PV
     with no transposes; softmax denominator via an all-ones stationary
     matmul; exp on the scalar engine (bf16 out); diagonal tiles masked
     post-exp with gpsimd affine_select (exact for the 0/-1e9 mask); score
     matmuls run LOOKAHEAD tiles ahead of PV.
  3. one output-projection pass at the end over the full-sequence o2 buffer:
     wo is streamed once (8.4MB instead of 33.6MB), 8-matmul chains per
     (n-tile, l-chunk), fp32 copy on the vector engine, DMA out.  wo and
     output DMAs alternate engines so no single DGE queue serializes the
     tail.

RoPE: wq/wk rows are pair-permuted ([evens|odds] per head) on the host so the
rotation becomes a partition half-swap folded into partition-offset operands
of the sin multiply; cs2/sn2 hold [cos|cos] and [-sin|sin] stacked.
"""

import numpy as np

import concourse.mybir as mybir
import concourse.tile as tile
from concourse import bacc, bass_utils

B, L, D = 2, 2048, 4096
H, HKV, HD = 32, 8, 128
NCORES = 8
GROUPS = 4                # head groups (cores per batch element)
QH = H // GROUPS          # 8 q heads per core
KVH = HKV // GROUPS       # 2 kv heads per core
NM = QH + 2 * KVH         # 12 projection m-tiles per core
LC = 512                  # l-chunk (matmul moving free dim)
KSUB = D // 128           # 32 contraction subtiles
SCALE = 1.0 / float(np.sqrt(HD))
LOOKAHEAD = 3             # score-matmul tiles in flight ahead of PV

f32 = mybir.dt.float32
bf16 = mybir.dt.bfloat16

# emission order: v/k first so their PSUM->SBUF stores land before attention
MI_ORDER = [QH + 2, QH + 3, QH, QH + 1] + list(range(QH))


def build_nc(seq_len=L):
    nlc = seq_len // LC
    nc = bacc.Bacc(trn_type="TRN2")

    x_tl = nc.dram_tensor("x_tl", [nlc * 4, 128, 8 * LC], bf16, kind="ExternalInput")
    wqkv_tl = nc.dram_tensor("wqkv_tl", [NM, 128, KSUB * 128], bf16, kind="ExternalInput")
    wo_tl = nc.dram_tensor("wo_tl", [D // 128, 128, QH * 128], bf16, kind="ExternalInput")
    cs2_d = nc.dram_tensor("cs2_d", [128, seq_len], bf16, kind="ExternalInput")
    sn2_d = nc.dram_tensor("sn2_d", [128, seq_len], bf16, kind="ExternalInput")
    ones_d = nc.dram_tensor("ones_d", [128, 128], bf16, kind="ExternalInput")
    ident_d = nc.dram_tensor("ident_d", [128, 128], bf16, kind="ExternalInput")
    outT = nc.dram_tensor("outT", [D, seq_len], bf16, kind="ExternalOutput")

    with tile.TileContext(nc) as tc:
        with (
            tc.tile_pool(name="persist", bufs=1) as pp,
            tc.tile_pool(name="xp", bufs=2) as xp,
            tc.tile_pool(name="wqp", bufs=2) as wqp,
            tc.tile_pool(name="qp", bufs=1) as qp,
            tc.tile_pool(name="wp", bufs=3) as wp,
            tc.tile_pool(name="ep", bufs=6) as ep,
            tc.tile_pool(name="tp", bufs=1) as tp,
            tc.tile_pool(name="outp", bufs=4) as outp,
            tc.tile_pool(name="mmps", bufs=4, space="PSUM") as mmps,
            tc.tile_pool(name="ops", bufs=2, space="PSUM") as ops_,
            tc.tile_pool(name="dps", bufs=2, space="PSUM") as dps,
        ):
            # ---- persistent SBUF state (k/v weights resident; wq streamed) ----
            wkv_s = pp.tile([128, 2 * KVH, KSUB * 128], bf16)
            kT_t = pp.tile([128, KVH, seq_len], bf16)
            v_t = pp.tile([128, seq_len // 128, KVH * HD], bf16)
            o2 = pp.tile([128, QH, seq_len], bf16)
            cs2 = pp.tile([128, seq_len], bf16)
            sn2 = pp.tile([128, seq_len], bf16)
            o128 = pp.tile([128, 128], bf16)
            idt = pp.tile([128, 128], bf16)

            # x quarters spread over the three DMA-issuing engines so each
            # lands on its own completion semaphore and no queue backs up
            dma_engines = [nc.sync, nc.scalar, nc.gpsimd, nc.sync]

            # split each resident k/v weight tile in half for DMA parallelism;
            # v weights first (first chains), k weights after chunk-0's x
            hw_ = KSUB * 128 // 2

            def fetch_wkv(kvms, engines):
                for i, kvm in enumerate(kvms):
                    for hf in range(2):
                        engines[2 * i + hf].dma_start(
                            wkv_s[:, kvm, hf * hw_:(hf + 1) * hw_],
                            wqkv_tl.ap()[QH + kvm][:, hf * hw_:(hf + 1) * hw_],
                        )

            # chunk-0 critical path: only v0 (the first chain's weights) goes
            # ahead of x; everything else queues behind the x quarters
            fetch_wkv((2,), [nc.sync, nc.scalar])

            # output-projection chains for finished l-chunks, interleaved into
            # the next chunk's attention as PE filler (see phase 2)
            pending_op = []
            wo_seq = [0]

            def emit_outproj(nt, slc):
                wo_t = wp.tile([128, QH, 128], bf16, tag="w", name=f"wo_{nt}_{slc}")
                eng = nc.gpsimd if wo_seq[0] % 2 == 0 else nc.sync
                wo_seq[0] += 1
                eng.dma_start(
                    wo_t[:], wo_tl.ap()[nt].rearrange("p (a b) -> p a b", a=QH)
                )
                sl = slice(slc * LC, (slc + 1) * LC)
                pso = mmps.tile([128, LC], f32, tag="mm")
                for h2 in range(QH):
                    nc.tensor.matmul(
                        pso[:], wo_t[:, h2, :], o2[:, h2, sl],
                        start=(h2 == 0), stop=(h2 == QH - 1),
                    )
                ob = outp.tile([128, LC], bf16, tag="ob")
                nc.vector.tensor_copy(ob[:], pso[:])
                (nc.sync if wo_seq[0] % 2 == 0 else nc.gpsimd).dma_start(
                    outT.ap()[nt * 128:(nt + 1) * 128, sl], ob[:]
                )

            for lc in range(nlc):
                lsl = slice(lc * LC, (lc + 1) * LC)
                x_q = [xp.tile([128, 8, LC], bf16, tag=f"x{qq}", name=f"x_{qq}")
                       for qq in range(4)]
                engines = (
                    [nc.sync, nc.scalar, nc.gpsimd, nc.gpsimd]
                    if lc == 0 else dma_engines
                )
                for qq in range(4):
                    engines[qq].dma_start(
                        x_q[qq][:],
                        x_tl.ap()[lc * 4 + qq].rearrange("p (a b) -> p a b", a=8),
                    )
                if lc == 0:
                    fetch_wkv((3,), [nc.gpsimd, nc.gpsimd])
                    fetch_wkv((0,), [nc.sync, nc.scalar])
                    fetch_wkv((1,), [nc.gpsimd, nc.sync])
                    nc.scalar.dma_start(cs2[:], cs2_d.ap())
                    nc.scalar.dma_start(sn2[:], sn2_d.ap())
                    nc.scalar.dma_start(o128[:], ones_d.ap())
                    nc.scalar.dma_start(idt[:], ident_d.ap())
                q_pr = qp.tile([128, QH, LC], bf16, tag="q")

                wq_tiles = {}

                def fetch_wq(m):
                    wqt = wqp.tile([128, KSUB * 128], bf16, tag="wq", name=f"wq_{m}")
                    nc.sync.dma_start(wqt[:], wqkv_tl.ap()[m])
                    wq_tiles[m] = wqt

                fetch_wq(0)
                fetch_wq(1)

                # ---- phase 1: projections, full-D chains (v/k first) ----
                for mi in MI_ORDER:
                    kind = "q" if mi < QH else ("k" if mi < QH + KVH else "v")
                    m = mi if mi < QH else (mi - QH if kind == "k" else mi - QH - KVH)
                    if kind == "q":
                        w_ap = wq_tiles.pop(m)[:].rearrange("p (a b) -> p a b", a=KSUB)
                    else:
                        kvm = mi - QH
                        w_ap = wkv_s[:, kvm, :].rearrange("p (a b) -> p a b", a=KSUB)
                    ps = mmps.tile([128, LC], f32, tag="mm")
                    for k in range(KSUB):
                        nc.tensor.matmul(
                            ps[:],
                            w_ap[:, k, :],
                            x_q[k // 8][:, k % 8, :],
                            start=(k == 0), stop=(k == KSUB - 1),
                        )
                    if kind == "q" and m + 2 < QH:
                        fetch_wq(m + 2)
                    if kind in ("q", "k"):
                        t1 = tp.tile([128, LC], f32, tag="t1")
                        nc.vector.tensor_mul(t1[:], ps[:], cs2[:, lsl])
                        t2 = tp.tile([128, LC], f32, tag="t2")
                        nc.vector.tensor_mul(
                            t2[0:64, :], ps[64:128, :], sn2[0:64, lsl]
                        )
                        nc.vector.tensor_mul(
                            t2[64:128, :], ps[0:64, :], sn2[64:128, lsl]
                        )
                        dst = q_pr[:, m, :] if kind == "q" else kT_t[:, m, lsl]
                        nc.vector.tensor_tensor(dst, t1[:], t2[:], mybir.AluOpType.add)
                    else:
                        vt = tp.tile([128, LC], bf16, tag="vt", bufs=2)
                        nc.scalar.copy(vt[:], ps[:])
                        for jj in range(4):
                            pt = mmps.tile([128, 128], bf16, tag="mm")
                            nc.tensor.transpose(
                                pt[:], vt[:, jj * 128:(jj + 1) * 128], idt[:]
                            )
                            nc.scalar.copy(
                                v_t[:, 4 * lc + jj, m * 128:(m + 1) * 128], pt[:]
                            )

                # ---- phase 2: causal attention for this chunk's queries ----
                # diagonal tiles (dg>=1) skip their leading 128*dg query
                # columns, which are masked for every key partition.  They are
                # processed FIRST so their exp+affine_select latency hides
                # under the initial lookahead burst instead of stalling the
                # tail of each head (PSUM has_written bits make out-of-order
                # accumulation with partial-width writes safe).
                njt = 4 * (lc + 1)
                jt_order = list(range(njt))
                if lc >= 1:
                    pending_op.extend((nt, lc - 1) for nt in range(D // 128))
                for h in range(QH):
                    kv = h // (QH // KVH)
                    po = ops_.tile([128, LC], f32, tag="po")
                    pden = dps.tile([128, LC], f32, tag="pden")
                    e_tiles = {}

                    def emit_score(jt, h=h, kv=kv, e_tiles=e_tiles, lc=lc):
                        dg = jt - 4 * lc
                        y0 = max(0, 128 * dg)
                        psS = mmps.tile([128, LC], f32, tag="mm")
                        nc.tensor.matmul(
                            psS[:, y0:],
                            kT_t[:, kv, jt * 128:(jt + 1) * 128],
                            q_pr[:, h, y0:],
                            start=True, stop=True,
                        )
                        e = ep.tile([128, LC], bf16, tag="e")
                        nc.scalar.activation(
                            e[:, y0:], psS[:, y0:],
                            mybir.ActivationFunctionType.Exp, scale=SCALE,
                        )
                        if dg >= 0:
                            # causal: zero E where key j > query l
                            nc.gpsimd.affine_select(
                                out=e[:, y0:], in_=e[:, y0:],
                                compare_op=mybir.AluOpType.is_ge,
                                fill=0.0,
                                base=0,
                                pattern=[[1, LC - y0]],
                                channel_multiplier=-1,
                            )
                        e_tiles[jt] = (e, y0)

                    for si in range(min(LOOKAHEAD, njt)):
                        emit_score(jt_order[si])
                    for si, jt in enumerate(jt_order):
                        if si + LOOKAHEAD < njt:
                            emit_score(jt_order[si + LOOKAHEAD])
                        e, y0 = e_tiles.pop(jt)
                        nc.tensor.matmul(
                            po[:, y0:],
                            v_t[:, jt, kv * 128:(kv + 1) * 128],
                            e[:, y0:],
                            start=(si == 0), stop=(si == njt - 1),
                        )
                        nc.tensor.matmul(
                            pden[:, y0:], o128[:], e[:, y0:],
                            start=(si == 0), stop=(si == njt - 1),
                        )
                        if si % 2 == 1 and pending_op:
                            emit_outproj(*pending_op.pop(0))
                    rec = tp.tile([128, LC], f32, tag="rec", bufs=2)
                    nc.vector.reciprocal_approx_fast(out=rec[:], in_=pden[:])
                    nc.vector.tensor_mul(o2[:, h, lsl], po[:], rec[:])

            # ---- phase 3: flush remaining output-projection chains ----
            pending_op.extend((nt, nlc - 1) for nt in range(D // 128))
            for nt, slc in pending_op:
                emit_outproj(nt, slc)
    nc.compile()
    return nc


_PERM = np.concatenate([np.arange(0, HD, 2), np.arange(1, HD, 2)])


def shard_inputs(x, wq, wk, wv, wo, cos, sin, mask, seq_len=L):
    """Build the 8 per-core input maps (host pre-tiling, bf16)."""
    import ml_dtypes

    nlc = seq_len // LC
    bf = ml_dtypes.bfloat16
    cosT = np.asarray(cos[:seq_len].T, np.float32)   # [64, seq]
    sinT = np.asarray(sin[:seq_len].T, np.float32)
    cs2 = np.ascontiguousarray(np.vstack([cosT, cosT]).astype(bf))
    sn2 = np.ascontiguousarray(np.vstack([-sinT, sinT]).astype(bf))
    ones128 = np.ones((128, 128), bf)
    ident = np.eye(128, dtype=bf)

    x_tls = []
    for b in range(B):
        xT = x[b, :seq_len].T.astype(bf)              # [D, seq]
        # x_tl[lc*4+qq][p, ks8*LC + y] = xT[(qq*8+ks8)*128+p, lc*LC+y]
        xv = xT.reshape(4, 8, 128, nlc, LC)           # [qq, ks8, p, lc, y]
        xv = xv.transpose(3, 0, 2, 1, 4)              # [lc, qq, p, ks8, y]
        x_tls.append(np.ascontiguousarray(xv.reshape(nlc * 4, 128, 8 * LC)))

    def permute_rows(w):
        nh = w.shape[0] // HD
        return w.reshape(nh, HD, -1)[:, _PERM, :].reshape(w.shape)

    in_maps = []
    for c in range(NCORES):
        b, g = divmod(c, GROUPS)
        wq_g = permute_rows(wq[QH * HD * g:QH * HD * (g + 1)])
        wk_g = permute_rows(wk[KVH * HD * g:KVH * HD * (g + 1)])
        wv_g = wv[KVH * HD * g:KVH * HD * (g + 1)]
        w_all = np.concatenate([wq_g, wk_g, wv_g], axis=0).astype(bf)  # [1536, D]
        # wqkv_tl[mi][p, k*128+mc] = w_all[mi*128+mc, k*128+p]
        wt = w_all.reshape(NM, 128, KSUB, 128)        # [mi, mc, k, p]
        wt = wt.transpose(0, 3, 2, 1)                 # [mi, p, k, mc]
        wqkv_tl = np.ascontiguousarray(wt.reshape(NM, 128, KSUB * 128))
        wo_g = wo[:, QH * HD * g:QH * HD * (g + 1)].astype(bf)  # [D(n), 1024(kk)]
        # wo_tl[nt][p, hs*128+nc] = wo_g[nt*128+nc, hs*128+p]
        wv_ = wo_g.reshape(D // 128, 128, QH, 128)    # [nt, nc, hs, p]
        wv_ = wv_.transpose(0, 3, 2, 1)               # [nt, p, hs, nc]
        wo_tl = np.ascontiguousarray(wv_.reshape(D // 128, 128, QH * 128))
        in_maps.append({
            "x_tl": x_tls[b],
            "wqkv_tl": wqkv_tl,
            "wo_tl": wo_tl,
            "cs2_d": cs2,
            "sn2_d": sn2,
            "ones_d": ones128,
            "ident_d": ident,
        })
    return in_maps


def gather_output(results, seq_len=L):
    out = np.zeros((B, seq_len, D), np.float32)
    for c in range(NCORES):
        b = c // GROUPS
        out[b] += results[c]["outT"].T.astype(np.float32)
    return out


_nc_cache = {}


def _get_nc(seq_len=L):
    if seq_len not in _nc_cache:
        _nc_cache[seq_len] = build_nc(seq_len)
    return _nc_cache[seq_len]


def run_sharded(inputs, trace=False, tmpdir=None):
    nc = _get_nc()
    in_maps = shard_inputs(**inputs)
    res = bass_utils.run_bass_kernel_spmd(
        nc, in_maps, core_ids=list(range(NCORES)), trace=trace, tmpdir=tmpdir
    )
    return gather_output(res.results), res


def kernel(**inputs) -> np.ndarray:
    out, _ = run_sharded(inputs)
    return out
